# revision 31
# baseline (speedup 1.0000x reference)
"""E3CoordLayer GNN message-passing kernel for 8 Trainium2 NeuronCores.

Strategy v6 (baseline loop structure + window-packed runs):
  - Edges sorted by row; core c owns rows [c*6250, (c+1)*6250). Runs of
    RUNW=1024 slots packed greedily: each run covers a 128-row window
    (host-chosen base b(c,r)); slots 0-511 hold col<25000 edges ("lo"),
    512-1023 hold col>=25000 ("hi"). ~1% padding vs 15% for the fixed
    block/TP scheme, so ~11% less of everything (gather descriptors,
    matmul columns, silu elements).
  - Gathers: xbar-transposed single-packet dma_gather from bf16 hlo/hhi
    tables, 2x512 per run on rotating SWDGE queues, with the validated
    GUARD_DIST=8 xbar-flush guard.
  - q = h_window @ W1a precomputed for ALL run windows before the main
    loop (host ships hTw = per-run 128-row hT windows), keeping the
    steady-state loop lean.
  - z1[f, slot] accumulates 3 streams per 512-chunk, grouped by weight:
    W1b^T hcol (bf16), W1c^T ea (fp8 rhs), q^T M (fp8 one-hot rhs,
    contraction = 128-row window rel index).
  - silu -> z2 = W2^T z1sb -> silu+b2 -> z3 per 128-slot tile -> tanh
    -> cdt = cdw*sc -> agg[3, 128] via 8 one-hot matmuls -> outT.
  - Host applies (x + overlap-summed window aggs) * flags on unshard.
"""
import sys
import os

sys.path.insert(0, "/opt/trn_rl_repo")

import numpy as np
import ml_dtypes

N_NODES = 50000
N_EDGES = 800000
HIDDEN = 128
EDGE_DIM = 16
COORDS_RANGE = 15.0
NCORES = 8
P = 128
NPC = N_NODES // NCORES          # 6250 nodes per core
W = 128                          # rows per run window
RUNW = 1024                      # slots per run
HALF = RUNW // 2                 # lo/hi slots per run
C0 = 25000                       # lo/hi table split
TP = RUNW // P                   # 8 tiles of 128 slots per run
RCH = 4                          # runs per streamed input chunk
ED1 = EDGE_DIM + 1               # edge-attr rows + ones row (b1)

_BF16 = ml_dtypes.bfloat16
_FP8 = ml_dtypes.float8_e4m3
GUARD_DIST = 8


def _wrap_idx(idx_call):
    """int16 index list [NI] -> [128, NI//16] (16-part wrap, replicated 8x)."""
    ni = idx_call.shape[0]
    w = idx_call.reshape(ni // 16, 16).T  # [16, NI//16]
    return np.tile(w, (8, 1))             # [128, NI//16]


def _build_nc(NRUNS):
    import concourse.bass as bass
    import concourse.mybir as mybir
    import concourse.tile as tile
    from concourse import bacc
    from concourse import library_config

    dt = mybir.dt
    S = NRUNS * RUNW
    NCH = (NRUNS + RCH - 1) // RCH

    nc = bacc.Bacc("TRN2", target_bir_lowering=False, debug=False,
                   num_devices=NCORES, num_swdge_queues=4,
                   dynamic_dma_scratch_size=65536)

    hlo = nc.dram_tensor("hlo", [C0 + P, HIDDEN], dt.bfloat16,
                         kind="ExternalInput")
    hhi = nc.dram_tensor("hhi", [N_NODES - C0 + P, HIDDEN], dt.bfloat16,
                         kind="ExternalInput")
    idxw = nc.dram_tensor("idxw", [P, S // 16], dt.int16, kind="ExternalInput")
    Mh = nc.dram_tensor("Mh", [P, S], dt.float8e4, kind="ExternalInput")
    ohh = nc.dram_tensor("ohh", [P, NRUNS * TP * P], dt.float8e4,
                         kind="ExternalInput")
    eaT = nc.dram_tensor("eaT", [ED1, S], dt.float8e4, kind="ExternalInput")
    cdw = nc.dram_tensor("cdw", [P, NRUNS * TP * 3], dt.bfloat16,
                         kind="ExternalInput")
    hTw = nc.dram_tensor("hTw", [P, NRUNS * P], dt.bfloat16,
                         kind="ExternalInput")
    w1a = nc.dram_tensor("w1a", [HIDDEN, HIDDEN], dt.bfloat16, kind="ExternalInput")
    w1b = nc.dram_tensor("w1b", [HIDDEN, HIDDEN], dt.bfloat16, kind="ExternalInput")
    w1c = nc.dram_tensor("w1c", [ED1, HIDDEN], dt.float8e4, kind="ExternalInput")
    w2 = nc.dram_tensor("w2", [HIDDEN, HIDDEN], dt.bfloat16, kind="ExternalInput")
    w3 = nc.dram_tensor("w3", [HIDDEN, 1], dt.bfloat16, kind="ExternalInput")
    b2 = nc.dram_tensor("b2", [HIDDEN, 1], dt.float32, kind="ExternalInput")
    outT = nc.dram_tensor("outT", [3, NRUNS * P], dt.float32,
                          kind="ExternalOutput")

    AF = mybir.ActivationFunctionType
    ALU = mybir.AluOpType

    with tile.TileContext(nc) as tc:
        nc.gpsimd.load_library(library_config.mlp)
        tc.strict_bb_all_engine_barrier()
        with (
            tc.tile_pool(name="const", bufs=1) as cp,
            tc.tile_pool(name="gath", bufs=GUARD_DIST + 4) as gp,
            tc.tile_pool(name="chunk", bufs=2) as chp,
            tc.tile_pool(name="work", bufs=2) as wp,
            tc.tile_pool(name="small", bufs=2) as scp,
            tc.tile_pool(name="zp", bufs=3, space="PSUM") as zp,
            tc.tile_pool(name="zq", bufs=1, space="PSUM") as zq,
            tc.tile_pool(name="pagg", bufs=1, space="PSUM") as pa,
        ):
            # ---- resident constants
            w1a_sb = cp.tile([HIDDEN, HIDDEN], dt.bfloat16)
            nc.sync.dma_start(out=w1a_sb[:], in_=w1a[:])
            w1b_sb = cp.tile([HIDDEN, HIDDEN], dt.bfloat16)
            nc.sync.dma_start(out=w1b_sb[:], in_=w1b[:])
            w1c_sb = cp.tile([ED1, HIDDEN], dt.float8e4)
            nc.sync.dma_start(out=w1c_sb[:], in_=w1c[:])
            w2_sb = cp.tile([HIDDEN, HIDDEN], dt.bfloat16)
            nc.sync.dma_start(out=w2_sb[:], in_=w2[:])
            w3_sb = cp.tile([HIDDEN, 1], dt.bfloat16)
            nc.sync.dma_start(out=w3_sb[:], in_=w3[:])
            b2_sb = cp.tile([HIDDEN, 1], dt.float32)
            nc.sync.dma_start(out=b2_sb[:], in_=b2[:])
            idx_sb = cp.tile([P, S // 16], dt.int16)
            nc.sync.dma_start(out=idx_sb[:], in_=idxw[:])
            cdw_sb = cp.tile([P, NRUNS, TP, 3], dt.bfloat16)
            nc.sync.dma_start(
                out=cdw_sb[:],
                in_=cdw[:].rearrange("p (r t c) -> p r t c", r=NRUNS, t=TP))
            hTw_sb = cp.tile([P, NRUNS * P], dt.bfloat16)
            nc.sync.dma_start(out=hTw_sb[:], in_=hTw[:])

            # ---- chunked inputs (M, oh, ea) with 1-chunk lookahead
            chunks = {}

            def fetch_chunk(k):
                if k >= NCH or k in chunks:
                    return
                c0, c1 = k * RCH * RUNW, (k + 1) * RCH * RUNW
                mch = chp.tile([P, RCH * RUNW], dt.float8e4, tag="M")
                nc.sync.dma_start(out=mch[:], in_=Mh[:, c0:c1])
                ohch = chp.tile([P, RCH * TP * P], dt.float8e4, tag="oh")
                nc.scalar.dma_start(
                    out=ohch[:], in_=ohh[:, k * RCH * TP * P:(k + 1) * RCH * TP * P])
                each = chp.tile([ED1, RCH * RUNW], dt.float8e4, tag="ea")
                nc.scalar.dma_start(out=each[:], in_=eaT[:, c0:c1])
                chunks[k] = (mch, ohch, each)

            fetch_chunk(0)
            fetch_chunk(1)

            from concourse.bass import _add_dep_helper
            z1b_by_run = {}
            hc_by_run = {}
            gcall = 0

            gi_by_run = {}

            def issue_gathers(r):
                e0 = r * RUNW
                hc = gp.tile([P, 1, RUNW], dt.bfloat16, tag="hc")
                gis = []
                for ci, htab in enumerate((hlo, hhi)):
                    gi = nc.gpsimd.dma_gather(
                        hc[:, :, ci * HALF:(ci + 1) * HALF],
                        htab[:],
                        idx_sb[:, (e0 + ci * HALF) // 16:(e0 + (ci + 1) * HALF) // 16],
                        HALF, HALF, HIDDEN, transpose=True,
                        queue_num=(2 * r + ci) % 4, single_packet=True,
                    )
                    gis.append(gi)
                    for prev in z1b_by_run.get(r - GUARD_DIST, ()):
                        _add_dep_helper(prev, gi.ins,
                                        reason="gather xbar-flush guard")
                hc_by_run[r] = hc
                gi_by_run[r] = gis
                return hc

            # Pool head start: gathers for the first runs only need idx_sb
            # and the DRAM tables, so issue them before the q precompute.
            PREF = GUARD_DIST + 4
            for r in range(min(PREF, NRUNS)):
                issue_gathers(r)

            # ---- q = h_window @ W1a for every run window, up-front
            q_sb = cp.tile([P, NRUNS, HIDDEN], dt.bfloat16)
            for r in range(NRUNS):
                qp = zp.tile([P, HIDDEN], dt.float32, tag="zp")
                nc.tensor.matmul(qp[:], lhsT=hTw_sb[:, r * P:(r + 1) * P],
                                 rhs=w1a_sb[:], start=True, stop=True)
                nc.vector.tensor_copy(out=q_sb[:, r, :], in_=qp[:])
            tc.strict_bb_all_engine_barrier()

            for r in range(NRUNS):
                k = r // RCH
                if r % RCH == 0:
                    fetch_chunk(k + 1)
                mch, ohch, each = chunks[k]
                roff = (r - k * RCH) * RUNW

                hc = hc_by_run.pop(r, None)
                if hc is None:
                    hc = issue_gathers(r)
                    del hc_by_run[r]

                # ---- z1 accumulation, matmuls grouped by weight
                z1p = zp.tile([P, RUNW], dt.float32, tag="zp")
                z1b_list = []
                for ci in range(2):
                    cs = slice(ci * HALF, (ci + 1) * HALF)
                    mm = nc.tensor.matmul(
                        z1p[:, cs], lhsT=w1b_sb[:],
                        rhs=hc[:, 0, cs],
                        start=True, stop=False)
                    z1b_list.append(mm.ins)
                z1b_by_run[r] = z1b_list
                if r + GUARD_DIST in gi_by_run:
                    for gi in gi_by_run[r + GUARD_DIST]:
                        for prev in z1b_list:
                            _add_dep_helper(prev, gi.ins,
                                            reason="gather xbar-flush guard")
                for ci in range(2):
                    cs = slice(ci * HALF, (ci + 1) * HALF)
                    nc.tensor.matmul(
                        z1p[:, cs], lhsT=w1c_sb[:],
                        rhs=each[:, roff + ci * HALF:roff + (ci + 1) * HALF],
                        start=False, stop=False)
                for ci in range(2):
                    cs = slice(ci * HALF, (ci + 1) * HALF)
                    nc.tensor.matmul(
                        z1p[:, cs], lhsT=q_sb[:, r, :],
                        rhs=mch[:, roff + ci * HALF:roff + (ci + 1) * HALF],
                        start=False, stop=True)

                # half-tile silus so z2/z3 matmuls overlap the second half
                z1sb0 = wp.tile([P, HALF], dt.bfloat16, tag="z1a")
                z1sb1 = wp.tile([P, HALF], dt.bfloat16, tag="z1b")
                z1sbs = [z1sb0, z1sb1]
                z2p = zp.tile([P, RUNW], dt.float32, tag="zp")
                for i in range(2):
                    cs = slice(i * HALF, (i + 1) * HALF)
                    nc.scalar.activation(out=z1sbs[i][:], in_=z1p[:, cs],
                                         func=AF.Silu)
                    nc.tensor.matmul(z2p[:, cs], lhsT=w2_sb[:],
                                     rhs=z1sbs[i][:], start=True, stop=True)
                z2sb0 = wp.tile([P, HALF], dt.bfloat16, tag="z2a")
                z2sb1 = wp.tile([P, HALF], dt.bfloat16, tag="z2b")
                z2sbs = [z2sb0, z2sb1]
                z3p = zq.tile([P, TP], dt.float32, tag="z3")
                for i in range(2):
                    cs = slice(i * HALF, (i + 1) * HALF)
                    nc.scalar.activation(out=z2sbs[i][:], in_=z2p[:, cs],
                                         func=AF.Silu, bias=b2_sb[:])
                    for tt in range(TP // 2):
                        t = i * TP // 2 + tt
                        nc.tensor.matmul(
                            z3p[:, t:t + 1],
                            lhsT=z2sbs[i][:, tt * P:(tt + 1) * P], rhs=w3_sb[:],
                            start=True, stop=True)

                sc = scp.tile([P, TP], dt.bfloat16, tag="sc")
                nc.scalar.activation(out=sc[:], in_=z3p[:], func=AF.Tanh)
                cdt = scp.tile([P, TP, 3], dt.bfloat16, tag="cdt")
                nc.vector.tensor_tensor(
                    out=cdt[:], in0=cdw_sb[:, r, :, :],
                    in1=sc[:].to_broadcast([P, TP, 3]), op=ALU.mult)
                aggp = pa.tile([3, P], dt.float32, tag="agg")
                for t in range(TP):
                    nc.tensor.matmul(
                        aggp[:], lhsT=cdt[:, t, :],
                        rhs=ohch[:, (r - k * RCH) * TP * P + t * P:
                                 (r - k * RCH) * TP * P + (t + 1) * P],
                        start=(t == 0), stop=(t == TP - 1))
                osb = scp.tile([3, P], dt.float32, tag="osb")
                nc.vector.tensor_copy(out=osb[:], in_=aggp[:])
                nc.sync.dma_start(out=outT[:, r * P:(r + 1) * P], in_=osb[:])
                done = (r + 1) // RCH - 1
                chunks.pop(done - 1, None)
    nc.compile()
    return nc


def _host_prep(h, edge_index, edge_attr, coord_diff):
    """Sort/pack edges into window runs; build per-core input maps.
    Returns (in_maps, NRUNS, bases) where bases[c] lists each run's
    window base row (relative to the core's first node)."""
    row = np.asarray(edge_index[0], dtype=np.int64)
    col = np.asarray(edge_index[1], dtype=np.int64)

    h32 = np.asarray(h, np.float32)
    h_bf = h32.astype(_BF16)
    hlo = np.zeros((C0 + P, HIDDEN), dtype=_BF16)
    hlo[:C0] = h_bf[:C0]
    hhi = np.zeros((N_NODES - C0 + P, HIDDEN), dtype=_BF16)
    hhi[:N_NODES - C0] = h_bf[C0:]
    hT = np.ascontiguousarray(h_bf.T)   # [128, N]

    ea = np.asarray(edge_attr, np.float32)
    cd15 = (np.asarray(coord_diff, np.float32) * COORDS_RANGE).astype(_BF16)

    # ---- pack runs per core
    packs = []
    for c in range(NCORES):
        m = (row // NPC) == c
        eidx = np.nonzero(m)[0]
        r = row[eidx] - c * NPC
        order = np.argsort(r, kind="stable")
        eidx = eidx[order]
        r = r[order]
        hi = col[eidx] >= C0
        lo_e, hi_e = eidx[~hi], eidx[hi]
        lo_r, hi_r = r[~hi], r[hi]
        il = ih = 0
        slots = []
        bases = []
        while il < len(lo_e) or ih < len(hi_e):
            nxt = []
            if il < len(lo_e):
                nxt.append(lo_r[il])
            if ih < len(hi_e):
                nxt.append(hi_r[ih])
            base = int(min(nxt))
            jl = min(int(np.searchsorted(lo_r, base + W)), il + HALF)
            jh = min(int(np.searchsorted(hi_r, base + W)), ih + HALF)
            sl = np.full(RUNW, -1, dtype=np.int64)
            sl[:jl - il] = lo_e[il:jl]
            sl[HALF:HALF + jh - ih] = hi_e[ih:jh]
            slots.append(sl)
            bases.append(base)
            il, ih = jl, jh
        packs.append((slots, bases))
    NRUNS = max(len(p[0]) for p in packs)
    NRUNS = ((NRUNS + RCH - 1) // RCH) * RCH   # uniform streamed chunks
    S = NRUNS * RUNW

    in_maps = []
    all_bases = []
    for c in range(NCORES):
        slots, bases = packs[c]
        while len(slots) < NRUNS:
            slots.append(np.full(RUNW, -1, dtype=np.int64))
            bases.append(0)
        sl = np.concatenate(slots)               # [S] edge id or -1
        valid = sl >= 0
        ei = np.where(valid, sl, 0)
        rel = np.where(
            valid,
            row[ei] - c * NPC - np.repeat(np.asarray(bases, np.int64), RUNW),
            -1)
        half = (np.arange(S) % RUNW) >= HALF

        idx = np.where(valid, col[ei] - half * C0, 0).astype(np.int16)
        idxw = np.zeros((P, S // 16), dtype=np.int16)
        for g in range(S // HALF):
            idxw[:, g * HALF // 16:(g + 1) * HALF // 16] = _wrap_idx(
                idx[g * HALF:(g + 1) * HALF])

        # M: [128 rel, S] one-hot
        Mm = np.zeros((P, S), dtype=_FP8)
        vs = np.nonzero(valid)[0]
        Mm[rel[vs], vs] = np.float32(1.0)
        # oh: [128, (r, t)*128 + rel]
        oh = np.zeros((P, S), dtype=_FP8)
        tix = np.arange(S) // P       # global tile index r*TP + t
        pix = np.arange(S) % P
        oh[pix[vs], tix[vs] * P + rel[vs]] = np.float32(1.0)
        # ea: [17, S]; rows 0..15 = ea dims, row 16 = ones (b1)
        eaT = np.zeros((ED1, S), dtype=_FP8)
        eaT[:EDGE_DIM, vs] = ea[ei[vs]].T.astype(_FP8)
        eaT[EDGE_DIM, vs] = np.float32(1.0)
        # cdw: [128, (r*TP + t)*3 + xyz]
        cdwc = np.zeros((P, NRUNS * TP * 3), dtype=_BF16)
        for x in range(3):
            cdwc[pix[vs], tix[vs] * 3 + x] = cd15[ei[vs], x]
        # hTw: window columns per run
        hTw = np.zeros((P, NRUNS * P), dtype=_BF16)
        n0 = c * NPC
        for r_ in range(NRUNS):
            b = bases[r_]
            wn = min(W, NPC - b)
            hTw[:, r_ * P:r_ * P + wn] = hT[:, n0 + b:n0 + b + wn]

        in_maps.append({
            "hlo": hlo, "hhi": hhi, "idxw": idxw, "Mh": Mm, "ohh": oh,
            "eaT": eaT, "cdw": cdwc, "hTw": hTw,
        })
        all_bases.append(bases)
    return in_maps, NRUNS, all_bases


def kernel(h, x, edge_index, edge_attr, coord_diff, flags, edge_mask,
           W1, b1, W2, b2, W3):
    from concourse.bass_utils import run_bass_kernel_spmd

    h = np.asarray(h, dtype=np.float32)
    x = np.asarray(x, dtype=np.float32)
    in_maps, NRUNS, all_bases = _host_prep(
        h, np.asarray(edge_index), np.asarray(edge_attr),
        np.asarray(coord_diff))

    W1 = np.asarray(W1, dtype=np.float32)
    w1c = np.zeros((ED1, HIDDEN), dtype=_FP8)
    w1c[:EDGE_DIM] = W1[2 * HIDDEN:].astype(_FP8)
    w1c[EDGE_DIM] = np.asarray(b1, dtype=np.float32).astype(_FP8)
    wshare = {
        "w1a": np.ascontiguousarray(W1[:HIDDEN].astype(_BF16)),
        "w1b": np.ascontiguousarray(W1[HIDDEN:2 * HIDDEN].astype(_BF16)),
        "w1c": w1c,
        "w2": np.ascontiguousarray(np.asarray(W2, np.float32).astype(_BF16)),
        "w3": np.ascontiguousarray(np.asarray(W3, np.float32).astype(_BF16)),
        "b2": np.asarray(b2, np.float32).reshape(HIDDEN, 1),
    }
    for m in in_maps:
        m.update(wshare)

    nc = _build_nc(NRUNS)
    res = run_bass_kernel_spmd(nc, in_maps, core_ids=list(range(NCORES)),
                               trace=os.environ.get("BASS_TRACE") == "1")
    global last_result
    last_result = res

    x = np.asarray(x, np.float32)
    out = x.copy()
    for c in range(NCORES):
        aggT = res.results[c]["outT"]          # [3, NRUNS*128]
        n0 = c * NPC
        for r, b in enumerate(all_bases[c]):
            wn = min(W, NPC - b)
            out[n0 + b:n0 + b + wn] += aggT[:, r * P:r * P + wn].T
    out *= np.asarray(flags, np.float32)
    return out


last_result = None


# revision 32
# speedup vs baseline: 1.1644x; 1.1644x over previous
"""E3CoordLayer GNN message-passing kernel for 8 Trainium2 NeuronCores.

Strategy v6 (baseline loop structure + window-packed runs):
  - Edges sorted by row; core c owns rows [c*6250, (c+1)*6250). Runs of
    RUNW=1024 slots packed greedily: each run covers a 128-row window
    (host-chosen base b(c,r)); slots 0-511 hold col<25000 edges ("lo"),
    512-1023 hold col>=25000 ("hi"). ~1% padding vs 15% for the fixed
    block/TP scheme, so ~11% less of everything (gather descriptors,
    matmul columns, silu elements).
  - Gathers: xbar-transposed single-packet dma_gather from bf16 hlo/hhi
    tables, 2x512 per run on rotating SWDGE queues, with the validated
    GUARD_DIST=8 xbar-flush guard.
  - q = h_window @ W1a precomputed for ALL run windows before the main
    loop (host ships hTw = per-run 128-row hT windows), keeping the
    steady-state loop lean.
  - z1[f, slot] accumulates 3 streams per 512-chunk, grouped by weight:
    W1b^T hcol (bf16), W1c^T ea (fp8 rhs), q^T M (fp8 one-hot rhs,
    contraction = 128-row window rel index).
  - silu -> z2 = W2^T z1sb -> silu+b2 -> z3 per 128-slot tile -> tanh
    -> cdt = cdw*sc -> agg[3, 128] via 8 one-hot matmuls -> outT.
  - Host applies (x + overlap-summed window aggs) * flags on unshard.
"""
import sys
import os

sys.path.insert(0, "/opt/trn_rl_repo")

import numpy as np
import ml_dtypes

N_NODES = 50000
N_EDGES = 800000
HIDDEN = 128
EDGE_DIM = 16
COORDS_RANGE = 15.0
NCORES = 8
P = 128
NPC = N_NODES // NCORES          # 6250 nodes per core
W = 128                          # rows per run window
RUNW = 1024                      # slots per run
HALF = RUNW // 2                 # lo/hi slots per run
C0 = 25000                       # lo/hi table split
TP = RUNW // P                   # 8 tiles of 128 slots per run
RCH = 4                          # runs per streamed input chunk
ED1 = EDGE_DIM + 1               # edge-attr rows + ones row (b1)

_BF16 = ml_dtypes.bfloat16
_FP8 = ml_dtypes.float8_e4m3
GUARD_DIST = 8


def _wrap_idx(idx_call):
    """int16 index list [NI] -> [128, NI//16] (16-part wrap, replicated 8x)."""
    ni = idx_call.shape[0]
    w = idx_call.reshape(ni // 16, 16).T  # [16, NI//16]
    return np.tile(w, (8, 1))             # [128, NI//16]


def _build_nc(NRUNS):
    import concourse.bass as bass
    import concourse.mybir as mybir
    import concourse.tile as tile
    from concourse import bacc
    from concourse import library_config

    dt = mybir.dt
    S = NRUNS * RUNW
    NCH = (NRUNS + RCH - 1) // RCH

    nc = bacc.Bacc("TRN2", target_bir_lowering=False, debug=False,
                   num_devices=NCORES, num_swdge_queues=4,
                   dynamic_dma_scratch_size=65536)

    hlo = nc.dram_tensor("hlo", [C0 + P, HIDDEN], dt.bfloat16,
                         kind="ExternalInput")
    hhi = nc.dram_tensor("hhi", [N_NODES - C0 + P, HIDDEN], dt.bfloat16,
                         kind="ExternalInput")
    idxw = nc.dram_tensor("idxw", [P, S // 16], dt.int16, kind="ExternalInput")
    Mh = nc.dram_tensor("Mh", [P, S], dt.float8e4, kind="ExternalInput")
    ohh = nc.dram_tensor("ohh", [P, NRUNS * TP * P], dt.float8e4,
                         kind="ExternalInput")
    eaT = nc.dram_tensor("eaT", [ED1, S], dt.float8e4, kind="ExternalInput")
    cdw = nc.dram_tensor("cdw", [P, NRUNS * TP * 3], dt.bfloat16,
                         kind="ExternalInput")
    hTw = nc.dram_tensor("hTw", [P, NRUNS * P], dt.bfloat16,
                         kind="ExternalInput")
    w1a = nc.dram_tensor("w1a", [HIDDEN, HIDDEN], dt.bfloat16, kind="ExternalInput")
    w1b = nc.dram_tensor("w1b", [HIDDEN, HIDDEN], dt.bfloat16, kind="ExternalInput")
    w1c = nc.dram_tensor("w1c", [ED1, HIDDEN], dt.float8e4, kind="ExternalInput")
    w2 = nc.dram_tensor("w2", [HIDDEN, HIDDEN], dt.bfloat16, kind="ExternalInput")
    w3 = nc.dram_tensor("w3", [HIDDEN, 1], dt.bfloat16, kind="ExternalInput")
    b2 = nc.dram_tensor("b2", [HIDDEN, 1], dt.float32, kind="ExternalInput")
    outT = nc.dram_tensor("outT", [3, NRUNS * P], dt.float32,
                          kind="ExternalOutput")

    AF = mybir.ActivationFunctionType
    ALU = mybir.AluOpType

    with tile.TileContext(nc) as tc:
        nc.gpsimd.load_library(library_config.mlp)
        tc.strict_bb_all_engine_barrier()
        with (
            tc.tile_pool(name="const", bufs=1) as cp,
            tc.tile_pool(name="gath", bufs=GUARD_DIST + 4) as gp,
            tc.tile_pool(name="chunk", bufs=2) as chp,
            tc.tile_pool(name="work", bufs=2) as wp,
            tc.tile_pool(name="small", bufs=2) as scp,
            tc.tile_pool(name="zp", bufs=3, space="PSUM") as zp,
            tc.tile_pool(name="zq", bufs=1, space="PSUM") as zq,
            tc.tile_pool(name="pagg", bufs=1, space="PSUM") as pa,
        ):
            # ---- resident constants
            w1a_sb = cp.tile([HIDDEN, HIDDEN], dt.bfloat16)
            nc.sync.dma_start(out=w1a_sb[:], in_=w1a[:])
            w1b_sb = cp.tile([HIDDEN, HIDDEN], dt.bfloat16)
            nc.sync.dma_start(out=w1b_sb[:], in_=w1b[:])
            w1c_sb = cp.tile([ED1, HIDDEN], dt.float8e4)
            nc.sync.dma_start(out=w1c_sb[:], in_=w1c[:])
            w2_sb = cp.tile([HIDDEN, HIDDEN], dt.bfloat16)
            nc.sync.dma_start(out=w2_sb[:], in_=w2[:])
            w3_sb = cp.tile([HIDDEN, 1], dt.bfloat16)
            nc.sync.dma_start(out=w3_sb[:], in_=w3[:])
            b2_sb = cp.tile([HIDDEN, 1], dt.float32)
            nc.sync.dma_start(out=b2_sb[:], in_=b2[:])
            idx_sb = cp.tile([P, S // 16], dt.int16)
            nc.sync.dma_start(out=idx_sb[:], in_=idxw[:])
            cdw_sb = cp.tile([P, NRUNS, TP, 3], dt.bfloat16)
            nc.sync.dma_start(
                out=cdw_sb[:],
                in_=cdw[:].rearrange("p (r t c) -> p r t c", r=NRUNS, t=TP))
            hTw_sb = cp.tile([P, NRUNS * P], dt.bfloat16)
            nc.sync.dma_start(out=hTw_sb[:], in_=hTw[:])

            # ---- chunked inputs (M, oh, ea) with 1-chunk lookahead
            chunks = {}

            def fetch_chunk(k):
                if k >= NCH or k in chunks:
                    return
                c0, c1 = k * RCH * RUNW, (k + 1) * RCH * RUNW
                mch = chp.tile([P, RCH * RUNW], dt.float8e4, tag="M")
                nc.sync.dma_start(out=mch[:], in_=Mh[:, c0:c1])
                ohch = chp.tile([P, RCH * TP * P], dt.float8e4, tag="oh")
                nc.scalar.dma_start(
                    out=ohch[:], in_=ohh[:, k * RCH * TP * P:(k + 1) * RCH * TP * P])
                each = chp.tile([ED1, RCH * RUNW], dt.float8e4, tag="ea")
                nc.scalar.dma_start(out=each[:], in_=eaT[:, c0:c1])
                chunks[k] = (mch, ohch, each)

            fetch_chunk(0)
            fetch_chunk(1)

            from concourse.bass import _add_dep_helper
            z1b_by_run = {}
            hc_by_run = {}
            gcall = 0

            gi_by_run = {}

            def issue_gathers(r):
                e0 = r * RUNW
                hc = gp.tile([P, 1, RUNW], dt.bfloat16, tag="hc")
                gis = []
                for ci, htab in enumerate((hlo, hhi)):
                    gi = nc.gpsimd.dma_gather(
                        hc[:, :, ci * HALF:(ci + 1) * HALF],
                        htab[:],
                        idx_sb[:, (e0 + ci * HALF) // 16:(e0 + (ci + 1) * HALF) // 16],
                        HALF, HALF, HIDDEN, transpose=True,
                        queue_num=(2 * r + ci) % 4, single_packet=True,
                    )
                    gis.append(gi)
                    for prev in z1b_by_run.get(r - GUARD_DIST, ()):
                        _add_dep_helper(prev, gi.ins,
                                        reason="gather xbar-flush guard")
                hc_by_run[r] = hc
                gi_by_run[r] = gis
                return hc

            # Pool head start: gathers for the first runs only need idx_sb
            # and the DRAM tables, so issue them before the q precompute.
            PREF = GUARD_DIST + 4
            for r in range(min(PREF, NRUNS)):
                issue_gathers(r)

            # ---- q = h_window @ W1a for every run window, up-front
            q_sb = cp.tile([P, NRUNS, HIDDEN], dt.bfloat16)
            for r in range(NRUNS):
                qp = zp.tile([P, HIDDEN], dt.float32, tag="zp")
                nc.tensor.matmul(qp[:], lhsT=hTw_sb[:, r * P:(r + 1) * P],
                                 rhs=w1a_sb[:], start=True, stop=True)
                nc.vector.tensor_copy(out=q_sb[:, r, :], in_=qp[:])
            tc.strict_bb_all_engine_barrier()

            for r in range(NRUNS):
                k = r // RCH
                if r % RCH == 0:
                    fetch_chunk(k + 1)
                mch, ohch, each = chunks[k]
                roff = (r - k * RCH) * RUNW

                hc = hc_by_run.pop(r, None)
                if hc is None:
                    hc = issue_gathers(r)
                    del hc_by_run[r]

                # ---- z1 accumulation, matmuls grouped by weight
                z1p = zp.tile([P, RUNW], dt.float32, tag="zp")
                z1b_list = []
                for ci in range(2):
                    cs = slice(ci * HALF, (ci + 1) * HALF)
                    mm = nc.tensor.matmul(
                        z1p[:, cs], lhsT=w1b_sb[:],
                        rhs=hc[:, 0, cs],
                        start=True, stop=False)
                    z1b_list.append(mm.ins)
                z1b_by_run[r] = z1b_list
                if r + GUARD_DIST in gi_by_run:
                    for gi in gi_by_run[r + GUARD_DIST]:
                        for prev in z1b_list:
                            _add_dep_helper(prev, gi.ins,
                                            reason="gather xbar-flush guard")
                for ci in range(2):
                    cs = slice(ci * HALF, (ci + 1) * HALF)
                    nc.tensor.matmul(
                        z1p[:, cs], lhsT=w1c_sb[:],
                        rhs=each[:, roff + ci * HALF:roff + (ci + 1) * HALF],
                        start=False, stop=False)
                for ci in range(2):
                    cs = slice(ci * HALF, (ci + 1) * HALF)
                    nc.tensor.matmul(
                        z1p[:, cs], lhsT=q_sb[:, r, :],
                        rhs=mch[:, roff + ci * HALF:roff + (ci + 1) * HALF],
                        start=False, stop=True)

                z1sb = wp.tile([P, RUNW], dt.bfloat16, tag="z1")
                nc.scalar.activation(out=z1sb[:], in_=z1p[:], func=AF.Silu)
                z2p = zp.tile([P, RUNW], dt.float32, tag="zp")
                for c0 in range(0, RUNW, HALF):
                    nc.tensor.matmul(z2p[:, c0:c0 + HALF], lhsT=w2_sb[:],
                                     rhs=z1sb[:, c0:c0 + HALF],
                                     start=True, stop=True)
                z2sb = wp.tile([P, RUNW], dt.bfloat16, tag="z2")
                nc.scalar.activation(out=z2sb[:], in_=z2p[:], func=AF.Silu,
                                     bias=b2_sb[:])
                z3p = zq.tile([P, TP], dt.float32, tag="z3")
                for t in range(TP):
                    nc.tensor.matmul(z3p[:, t:t + 1],
                                     lhsT=z2sb[:, t * P:(t + 1) * P], rhs=w3_sb[:],
                                     start=True, stop=True)

                sc = scp.tile([P, TP], dt.bfloat16, tag="sc")
                nc.scalar.activation(out=sc[:], in_=z3p[:], func=AF.Tanh)
                cdt = scp.tile([P, TP, 3], dt.bfloat16, tag="cdt")
                nc.vector.tensor_tensor(
                    out=cdt[:], in0=cdw_sb[:, r, :, :],
                    in1=sc[:].to_broadcast([P, TP, 3]), op=ALU.mult)
                aggp = pa.tile([3, P], dt.float32, tag="agg")
                for t in range(TP):
                    nc.tensor.matmul(
                        aggp[:], lhsT=cdt[:, t, :],
                        rhs=ohch[:, (r - k * RCH) * TP * P + t * P:
                                 (r - k * RCH) * TP * P + (t + 1) * P],
                        start=(t == 0), stop=(t == TP - 1))
                osb = scp.tile([3, P], dt.float32, tag="osb")
                nc.vector.tensor_copy(out=osb[:], in_=aggp[:])
                nc.sync.dma_start(out=outT[:, r * P:(r + 1) * P], in_=osb[:])
                done = (r + 1) // RCH - 1
                chunks.pop(done - 1, None)
    nc.compile()
    return nc


def _host_prep(h, edge_index, edge_attr, coord_diff):
    """Sort/pack edges into window runs; build per-core input maps.
    Returns (in_maps, NRUNS, bases) where bases[c] lists each run's
    window base row (relative to the core's first node)."""
    row = np.asarray(edge_index[0], dtype=np.int64)
    col = np.asarray(edge_index[1], dtype=np.int64)

    h32 = np.asarray(h, np.float32)
    h_bf = h32.astype(_BF16)
    hlo = np.zeros((C0 + P, HIDDEN), dtype=_BF16)
    hlo[:C0] = h_bf[:C0]
    hhi = np.zeros((N_NODES - C0 + P, HIDDEN), dtype=_BF16)
    hhi[:N_NODES - C0] = h_bf[C0:]
    hT = np.ascontiguousarray(h_bf.T)   # [128, N]

    ea = np.asarray(edge_attr, np.float32)
    cd15 = (np.asarray(coord_diff, np.float32) * COORDS_RANGE).astype(_BF16)

    # ---- pack runs per core
    packs = []
    for c in range(NCORES):
        m = (row // NPC) == c
        eidx = np.nonzero(m)[0]
        r = row[eidx] - c * NPC
        order = np.argsort(r, kind="stable")
        eidx = eidx[order]
        r = r[order]
        hi = col[eidx] >= C0
        lo_e, hi_e = eidx[~hi], eidx[hi]
        lo_r, hi_r = r[~hi], r[hi]
        il = ih = 0
        slots = []
        bases = []
        while il < len(lo_e) or ih < len(hi_e):
            nxt = []
            if il < len(lo_e):
                nxt.append(lo_r[il])
            if ih < len(hi_e):
                nxt.append(hi_r[ih])
            base = int(min(nxt))
            jl = min(int(np.searchsorted(lo_r, base + W)), il + HALF)
            jh = min(int(np.searchsorted(hi_r, base + W)), ih + HALF)
            sl = np.full(RUNW, -1, dtype=np.int64)
            sl[:jl - il] = lo_e[il:jl]
            sl[HALF:HALF + jh - ih] = hi_e[ih:jh]
            slots.append(sl)
            bases.append(base)
            il, ih = jl, jh
        packs.append((slots, bases))
    NRUNS = max(len(p[0]) for p in packs)
    NRUNS = ((NRUNS + RCH - 1) // RCH) * RCH   # uniform streamed chunks
    S = NRUNS * RUNW

    in_maps = []
    all_bases = []
    for c in range(NCORES):
        slots, bases = packs[c]
        while len(slots) < NRUNS:
            slots.append(np.full(RUNW, -1, dtype=np.int64))
            bases.append(0)
        sl = np.concatenate(slots)               # [S] edge id or -1
        valid = sl >= 0
        ei = np.where(valid, sl, 0)
        rel = np.where(
            valid,
            row[ei] - c * NPC - np.repeat(np.asarray(bases, np.int64), RUNW),
            -1)
        half = (np.arange(S) % RUNW) >= HALF

        idx = np.where(valid, col[ei] - half * C0, 0).astype(np.int16)
        idxw = np.zeros((P, S // 16), dtype=np.int16)
        for g in range(S // HALF):
            idxw[:, g * HALF // 16:(g + 1) * HALF // 16] = _wrap_idx(
                idx[g * HALF:(g + 1) * HALF])

        # M: [128 rel, S] one-hot
        Mm = np.zeros((P, S), dtype=_FP8)
        vs = np.nonzero(valid)[0]
        Mm[rel[vs], vs] = np.float32(1.0)
        # oh: [128, (r, t)*128 + rel]
        oh = np.zeros((P, S), dtype=_FP8)
        tix = np.arange(S) // P       # global tile index r*TP + t
        pix = np.arange(S) % P
        oh[pix[vs], tix[vs] * P + rel[vs]] = np.float32(1.0)
        # ea: [17, S]; rows 0..15 = ea dims, row 16 = ones (b1)
        eaT = np.zeros((ED1, S), dtype=_FP8)
        eaT[:EDGE_DIM, vs] = ea[ei[vs]].T.astype(_FP8)
        eaT[EDGE_DIM, vs] = np.float32(1.0)
        # cdw: [128, (r*TP + t)*3 + xyz]
        cdwc = np.zeros((P, NRUNS * TP * 3), dtype=_BF16)
        for x in range(3):
            cdwc[pix[vs], tix[vs] * 3 + x] = cd15[ei[vs], x]
        # hTw: window columns per run
        hTw = np.zeros((P, NRUNS * P), dtype=_BF16)
        n0 = c * NPC
        for r_ in range(NRUNS):
            b = bases[r_]
            wn = min(W, NPC - b)
            hTw[:, r_ * P:r_ * P + wn] = hT[:, n0 + b:n0 + b + wn]

        in_maps.append({
            "hlo": hlo, "hhi": hhi, "idxw": idxw, "Mh": Mm, "ohh": oh,
            "eaT": eaT, "cdw": cdwc, "hTw": hTw,
        })
        all_bases.append(bases)
    return in_maps, NRUNS, all_bases


def kernel(h, x, edge_index, edge_attr, coord_diff, flags, edge_mask,
           W1, b1, W2, b2, W3):
    from concourse.bass_utils import run_bass_kernel_spmd

    h = np.asarray(h, dtype=np.float32)
    x = np.asarray(x, dtype=np.float32)
    in_maps, NRUNS, all_bases = _host_prep(
        h, np.asarray(edge_index), np.asarray(edge_attr),
        np.asarray(coord_diff))

    W1 = np.asarray(W1, dtype=np.float32)
    w1c = np.zeros((ED1, HIDDEN), dtype=_FP8)
    w1c[:EDGE_DIM] = W1[2 * HIDDEN:].astype(_FP8)
    w1c[EDGE_DIM] = np.asarray(b1, dtype=np.float32).astype(_FP8)
    wshare = {
        "w1a": np.ascontiguousarray(W1[:HIDDEN].astype(_BF16)),
        "w1b": np.ascontiguousarray(W1[HIDDEN:2 * HIDDEN].astype(_BF16)),
        "w1c": w1c,
        "w2": np.ascontiguousarray(np.asarray(W2, np.float32).astype(_BF16)),
        "w3": np.ascontiguousarray(np.asarray(W3, np.float32).astype(_BF16)),
        "b2": np.asarray(b2, np.float32).reshape(HIDDEN, 1),
    }
    for m in in_maps:
        m.update(wshare)

    nc = _build_nc(NRUNS)
    res = run_bass_kernel_spmd(nc, in_maps, core_ids=list(range(NCORES)),
                               trace=os.environ.get("BASS_TRACE") == "1")
    global last_result
    last_result = res

    x = np.asarray(x, np.float32)
    out = x.copy()
    for c in range(NCORES):
        aggT = res.results[c]["outT"]          # [3, NRUNS*128]
        n0 = c * NPC
        for r, b in enumerate(all_bases[c]):
            wn = min(W, NPC - b)
            out[n0 + b:n0 + b + wn] += aggT[:, r * P:r * P + wn].T
    out *= np.asarray(flags, np.float32)
    return out


last_result = None
